# revision 5
# baseline (speedup 1.0000x reference)
"""MoE (top-2 of 8 experts) Trainium2 Bass kernel.

Strategy (V1): data-parallel dense. 8192 tokens split across 8 cores
(1024 each). Each core:
  - transposes its x block on the PE (fp32) -> xT [d-part, tok]
  - fp32 router: hT = tanh(rw1.T @ xT + rb1); logits = hT.T @ rw2 + rb2
  - exact top-2 selection on LOGITS (index-masked argmax; replicates
    jax.lax.top_k semantics incl. ties), renormalized weights via
    sigmoid((p1-p2)/Z) computed from softmax probs
  - dense expert GEMMs in fp32r (full PE rate), per-token scaled
    accumulation into SBUF acc via scalar_tensor_tensor
  - bias term comb @ eb via a K=8 matmul that initializes the accumulator
"""

import sys

if "/opt/trn_rl_repo" not in sys.path:
    sys.path.insert(0, "/opt/trn_rl_repo")

import numpy as np

import concourse.bacc as bacc
import concourse.mybir as mybir
import concourse.tile as tile
from concourse.bass import ds, ts
from concourse.bass_utils import run_bass_kernel_spmd
from concourse.masks import make_identity

F32 = mybir.dt.float32
F32R = mybir.dt.float32r
AF = mybir.ActivationFunctionType
OP = mybir.AluOpType
X = mybir.AxisListType.X

B, D_IN, D_OUT, E, K, RH = 8192, 1024, 1024, 8, 2, 128
N_CORES = 8
T = B // N_CORES          # tokens per core
NT = T // 128             # token tiles per core (8)
NC_D = D_IN // 128        # d chunks (8)
NH = D_OUT // 512         # psum halves (2)


def build():
    nc = bacc.Bacc("TRN2", target_bir_lowering=False)

    x_d = nc.dram_tensor("x", [128, NT, D_IN], F32, kind="ExternalInput")
    rw1_d = nc.dram_tensor("rw1", [128, NC_D, RH], F32, kind="ExternalInput")
    rb1_d = nc.dram_tensor("rb1", [RH, 1], F32, kind="ExternalInput")
    rw2_d = nc.dram_tensor("rw2", [RH, E], F32, kind="ExternalInput")
    rb2_d = nc.dram_tensor("rb2", [1, E], F32, kind="ExternalInput")
    ew_d = nc.dram_tensor("ew", [E, D_IN, D_OUT], F32, kind="ExternalInput")
    eb_d = nc.dram_tensor("eb", [E, D_OUT], F32, kind="ExternalInput")
    out_d = nc.dram_tensor("out", [128, NT, D_OUT], F32, kind="ExternalOutput")

    with tile.TileContext(nc) as tc:
        with (
            tc.tile_pool(name="const", bufs=1) as const,
            tc.tile_pool(name="work", bufs=1) as work,
            tc.tile_pool(name="ewpool", bufs=2) as ewpool,
            tc.tile_pool(name="ptr", bufs=2, space="PSUM") as ptr,
            tc.tile_pool(name="prt", bufs=2, space="PSUM") as prt,
            tc.tile_pool(name="pexp", bufs=4, space="PSUM") as pexp,
        ):
            # ---- constants / weights ----
            ident = const.tile([128, 128], F32, tag="ident")
            make_identity(nc, ident[:])
            ones = const.tile([1, 128], F32, tag="ones")
            nc.vector.memset(ones[:], 1.0)
            iota_i = const.tile([128, E], mybir.dt.int32, tag="iota_i")
            nc.gpsimd.iota(iota_i[:], pattern=[[1, E]], base=0, channel_multiplier=0)
            iota_f = const.tile([128, E], F32, tag="iota_f")
            nc.vector.tensor_copy(iota_f[:], iota_i[:])
            iota_1k = const.tile([128, E], F32, tag="iota_1k")
            nc.vector.tensor_scalar_add(iota_1k[:], iota_f[:], 1000.0)

            rw1_sb = const.tile([128, NC_D, RH], F32, tag="rw1")
            nc.sync.dma_start(rw1_sb[:], rw1_d[:])
            rb1_sb = const.tile([RH, 1], F32, tag="rb1")
            nc.sync.dma_start(rb1_sb[:], rb1_d[:])
            rw2_sb = const.tile([RH, E], F32, tag="rw2")
            nc.sync.dma_start(rw2_sb[:], rw2_d[:])
            rb2_sb = const.tile([1, E], F32, tag="rb2")
            nc.sync.dma_start(rb2_sb[:], rb2_d[:])
            eb_sb = const.tile([E, D_OUT], F32, tag="eb")
            nc.sync.dma_start(eb_sb[:], eb_d[:])

            x_sb = work.tile([128, NT, D_IN], F32, tag="x")
            nc.sync.dma_start(x_sb[:], x_d[:])

            xT = work.tile([128, NC_D, T], F32, tag="xT")
            xT_r = work.tile([128, NC_D, T], F32R, tag="x")  # reuses x_sb slot after transposes
            hT = work.tile([128, NT, RH], F32, tag="hT")
            comb = work.tile([128, NT, E], F32, tag="comb")
            combT = work.tile([E, NT, 128], F32, tag="combT")
            acc = work.tile([128, NT, D_OUT], F32, tag="acc")

            # ---- transpose x, router, selection, per tile ----
            for t in range(NT):
                for c in range(NC_D):
                    pt = ptr.tile([128, 128], F32, tag="ptr")
                    nc.tensor.transpose(pt[:], x_sb[:, t, ts(c, 128)], ident[:])
                    nc.any.tensor_copy(xT[:, c, ts(t, 128)], pt[:])

                # hT tile: [RH, 128tok] = sum_c rw1_c.T @ xT_c
                ph = prt.tile([128, 128], F32, tag="prt")
                for c in range(NC_D):
                    nc.tensor.matmul(
                        ph[:], rw1_sb[:, c, :], xT[:, c, ts(t, 128)],
                        start=(c == 0), stop=(c == NC_D - 1),
                    )
                nc.scalar.activation(
                    hT[:, t, :],
                    ph[:], AF.Tanh, bias=rb1_sb[:, 0:1], scale=1.0,
                )

                # logits tile: [128tok, E] = hT_t.T @ rw2 + ones.T @ rb2
                pl = prt.tile([128, E], F32, tag="prt")
                nc.tensor.matmul(pl[:], hT[:, t, :], rw2_sb[:], start=True, stop=False)
                nc.tensor.matmul(pl[:], ones[:], rb2_sb[:], start=False, stop=True)

                lg = work.tile([128, E], F32, tag="lg")
                nc.any.tensor_copy(lg[:], pl[:])

                # exact top-2 on logits (first-argmax semantics, ties -> lower idx)
                l1 = work.tile([128, 1], F32, tag="l1")
                nc.vector.tensor_reduce(l1[:], lg[:], X, OP.max)
                eq1 = work.tile([128, E], F32, tag="eq1")
                nc.vector.tensor_scalar(eq1[:], lg[:], l1[:, 0:1], None, OP.is_equal)
                tmp1 = work.tile([128, E], F32, tag="tmp1")
                nc.vector.scalar_tensor_tensor(
                    tmp1[:], eq1[:], -1000.0, iota_1k[:], op0=OP.mult, op1=OP.add)
                e1f = work.tile([128, 1], F32, tag="e1f")
                nc.vector.tensor_reduce(e1f[:], tmp1[:], X, OP.min)
                oh1 = work.tile([128, E], F32, tag="oh1")
                nc.vector.tensor_scalar(oh1[:], iota_f[:], e1f[:, 0:1], None, OP.is_equal)

                lg2 = work.tile([128, E], F32, tag="lg2")
                nc.vector.scalar_tensor_tensor(
                    lg2[:], oh1[:], -100000.0, lg[:], op0=OP.mult, op1=OP.add)
                l2 = work.tile([128, 1], F32, tag="l2")
                nc.vector.tensor_reduce(l2[:], lg2[:], X, OP.max)
                eq2 = work.tile([128, E], F32, tag="eq2")
                nc.vector.tensor_scalar(eq2[:], lg2[:], l2[:, 0:1], None, OP.is_equal)
                tmp2 = work.tile([128, E], F32, tag="tmp2")
                nc.vector.scalar_tensor_tensor(
                    tmp2[:], eq2[:], -1000.0, iota_1k[:], op0=OP.mult, op1=OP.add)
                e2f = work.tile([128, 1], F32, tag="e2f")
                nc.vector.tensor_reduce(e2f[:], tmp2[:], X, OP.min)
                oh2 = work.tile([128, E], F32, tag="oh2")
                nc.vector.tensor_scalar(oh2[:], iota_f[:], e2f[:, 0:1], None, OP.is_equal)

                # softmax pieces: ex = exp(lg - max); Z = sum(ex)
                nl1 = work.tile([128, 1], F32, tag="nl1")
                nc.vector.tensor_scalar_mul(nl1[:], l1[:], -1.0)
                ex = work.tile([128, E], F32, tag="ex")
                nc.scalar.activation(ex[:], lg[:], AF.Exp, bias=nl1[:, 0:1], scale=1.0)
                zs = work.tile([128, 1], F32, tag="zs")
                nc.vector.tensor_reduce(zs[:], ex[:], X, OP.add)
                rz = work.tile([128, 1], F32, tag="rz")
                nc.vector.reciprocal(rz[:], zs[:])

                # p1raw/p2raw then w1 = sigmoid((p1raw-p2raw)/Z), w2 = 1-w1
                m1 = work.tile([128, E], F32, tag="m1")
                nc.vector.tensor_mul(m1[:], ex[:], oh1[:])
                p1r = work.tile([128, 1], F32, tag="p1r")
                nc.vector.tensor_reduce(p1r[:], m1[:], X, OP.add)
                m2 = work.tile([128, E], F32, tag="m2")
                nc.vector.tensor_mul(m2[:], ex[:], oh2[:])
                p2r = work.tile([128, 1], F32, tag="p2r")
                nc.vector.tensor_reduce(p2r[:], m2[:], X, OP.add)
                dp = work.tile([128, 1], F32, tag="dp")
                nc.vector.tensor_sub(dp[:], p1r[:], p2r[:])
                w1 = work.tile([128, 1], F32, tag="w1")
                nc.scalar.activation(w1[:], dp[:], AF.Sigmoid, bias=0.0, scale=rz[:, 0:1])
                w2 = work.tile([128, 1], F32, tag="w2")
                nc.vector.tensor_scalar(w2[:], w1[:], -1.0, 1.0, OP.mult, OP.add)

                # comb tile = oh1*w1 + oh2*w2  (note iota_1k-based onehots are exact)
                cm1 = work.tile([128, E], F32, tag="cm1")
                nc.vector.tensor_scalar_mul(cm1[:], oh1[:], w1[:, 0:1])
                nc.vector.scalar_tensor_tensor(
                    comb[:, t, :], oh2[:], w2[:, 0:1], cm1[:], op0=OP.mult, op1=OP.add)

                # combT tile for the bias matmul
                pc = prt.tile([E, 128], F32, tag="prt")
                nc.tensor.transpose(pc[:], comb[:, t, :], ident[:])
                nc.any.tensor_copy(combT[:, t, :], pc[:])

                # init acc with bias: acc_t = combT_t.T @ eb
                for nh in range(NH):
                    pb = pexp.tile([128, 512], F32, tag="pexp")
                    nc.tensor.matmul(
                        pb[:], combT[:, t, :], eb_sb[:, ts(nh, 512)],
                        start=True, stop=True,
                    )
                    nc.any.tensor_copy(acc[:, t, ts(nh, 512)], pb[:])

            # ---- dense expert GEMMs, fp32r ----
            nc.vector.tensor_copy(xT_r[:], xT[:])
            for e in range(E):
                ew_sb = ewpool.tile([128, NC_D, D_OUT], F32R, tag="ew")
                nc.gpsimd.dma_start(
                    ew_sb[:], ew_d[e].rearrange("(c p) n -> p c n", p=128))
                for t in range(NT):
                    for nh in range(NH):
                        pe_ = pexp.tile([128, 512], F32, tag="pexp")
                        for c in range(NC_D):
                            nc.tensor.matmul(
                                pe_[:],
                                xT_r[:, c, ts(t, 128)],
                                ew_sb[:, c, ts(nh, 512)],
                                start=(c == 0), stop=(c == NC_D - 1),
                            )
                        nc.vector.scalar_tensor_tensor(
                            acc[:, t, ts(nh, 512)], pe_[:], comb[:, t, e : e + 1],
                            acc[:, t, ts(nh, 512)], op0=OP.mult, op1=OP.add)

            for t in range(NT):
                nc.sync.dma_start(out_d[:, t, :], acc[:, t, :])

    nc.compile()
    return nc


_NC_CACHE = None


def _get_nc():
    global _NC_CACHE
    if _NC_CACHE is None:
        _NC_CACHE = build()
    return _NC_CACHE


def make_in_maps(x, rw1, rb1, rw2, rb2, ew, eb):
    x = np.ascontiguousarray(np.asarray(x, dtype=np.float32))
    rw1 = np.asarray(rw1, np.float32)
    shared = {
        "rw1": np.ascontiguousarray(
            rw1.reshape(NC_D, 128, RH).transpose(1, 0, 2)),
        "rb1": np.ascontiguousarray(np.asarray(rb1, np.float32).reshape(RH, 1)),
        "rw2": np.ascontiguousarray(np.asarray(rw2, np.float32)),
        "rb2": np.ascontiguousarray(np.asarray(rb2, np.float32).reshape(1, E)),
        "ew": np.ascontiguousarray(np.asarray(ew, np.float32)),
        "eb": np.ascontiguousarray(np.asarray(eb, np.float32)),
    }
    in_maps = []
    for c in range(N_CORES):
        xc = x[c * T : (c + 1) * T]  # [1024, 1024]
        xc = np.ascontiguousarray(xc.reshape(NT, 128, D_IN).transpose(1, 0, 2))
        in_maps.append({"x": xc, **shared})
    return in_maps


def assemble(results):
    outs = []
    for c in range(N_CORES):
        o = results[c]["out"]  # [128, NT, D_OUT]
        outs.append(np.ascontiguousarray(o.transpose(1, 0, 2)).reshape(T, D_OUT))
    return np.concatenate(outs, axis=0)


def run(inputs, trace=False, **kw):
    nc = _get_nc()
    in_maps = make_in_maps(**inputs)
    res = run_bass_kernel_spmd(
        nc, in_maps, core_ids=list(range(N_CORES)), trace=trace, **kw)
    return assemble(res.results), res


def kernel(**inputs) -> np.ndarray:
    out, _ = run(inputs, trace=False)
    return out
